# revision 50
# baseline (speedup 1.0000x reference)
"""BitLinear (BitNet-style) kernel for 8 Trainium2 NeuronCores.

Computes: out = input @ (sign(W) * mean(|W|)).T + bias
  input [8192, 2048] f32, W [8192, 2048] f32, bias [8192] f32 -> out [8192, 8192] f32

Sharding: column-parallel over out_features. Core j owns W rows
[j*1024, (j+1)*1024). Each core computes sign() on its shard (scalar
engine) and a local |W| partial sum (vector engine reduce with absolute
value); partial sums are AllReduce'd across the 8 cores so the scale is
the global abs-mean. The GEMM runs in bf16 (sign(W) is exactly
representable; input/weights are rounded host-side), accumulating in
fp32 PSUM. scale (fp32) and bias (fp32) are fused into the PSUM->SBUF
eviction: out = psum * scale + bias.

Layout: host ships input already transposed (inT = input.T, bf16) and
the weight shard transposed (wT = W.T shard, bf16) so both GEMM operands
are K-major as the tensor engine requires; each core writes its out.T
shard [1024, 8192] contiguously and the host re-transposes once.

Perf notes (cost-model + real-HW repeat-slope profiled):
- 2048 matmuls of [K=128]x[M=128 o]x[N=512 t] stream at ~214 ns each —
  the bf16 1-col/cycle floor (~438 us busy); projection ~450 us/core.
- Stationary sign-weights are fp8e4 (+-1 exact): on real HW this removed
  ~90 us/iter of exposed LDWEIGHTS time vs a bf16 stationary (measured
  539 -> 446 us/iter via R-repeat wall-clock slope), since every matmul
  carries its own weight load and bf16 FWL loads don't fully hide.
- Output stores issue on the ACT HWDGE ring so they can't head-of-line
  block the next span's input loads on the SP ring (strict per-ring FIFO).
- The scale chain never touches the in-order PE queue (cross-partition
  sum via DRAM bounce + DVE, broadcast via step-0 DMA), and its small
  DMAs stay off the SP HWDGE FIFO so they can't head-of-line block the
  input loads while waiting on the collective.
- Ramped token spans (512,512,1024,2048x3): early spans use 1 PSUM bank
  per o-group (up to 8 in flight) and a copy-only eviction with the
  scale/bias folded in a second DVE pass, so nothing stalls on the
  AllReduce latency.
"""

import sys

for _p in ("/opt/trn_rl_repo",):
    if _p not in sys.path:
        sys.path.append(_p)

import ml_dtypes
import numpy as np

TOKENS = 8192
D_IN = 2048
D_OUT = 8192
NCORES = 8
OSH = D_OUT // NCORES  # 1024 out features per core
P = 128
KT = D_IN // P         # 16 k-tiles of 128
TQ = 2048              # resident token span
OT = OSH // P          # 8 o-tiles per core
SPAN_SCHEDULE = (512, 512, 1024, 2048, 2048, 2048)

_NC_CACHE = {}


def _build_nc(use_collective=True, repeat=1, dedup_ldw=True):
    import concourse.mybir as mybir
    import concourse.tile as tile
    from concourse import bacc

    f32 = mybir.dt.float32
    bf16 = mybir.dt.bfloat16
    fp8 = mybir.dt.float8e4
    AF = mybir.ActivationFunctionType

    nc = bacc.Bacc("TRN2", target_bir_lowering=False, debug=False,
                   num_devices=NCORES)

    inT = nc.dram_tensor("inT", [D_IN, TOKENS], bf16, kind="ExternalInput")
    wT = nc.dram_tensor("wT", [D_IN, OSH], bf16, kind="ExternalInput")
    bias2d = nc.dram_tensor("bias2d", [P, OT], f32, kind="ExternalInput")
    outT = nc.dram_tensor("outT", [OSH, TOKENS], f32, kind="ExternalOutput")
    cc_in = nc.dram_tensor("cc_in", [1, 8], f32)
    cc_out = nc.dram_tensor("cc_out", [1, 8], f32, addr_space="Shared")
    colsum_dram = nc.dram_tensor("colsum_dram", [P], f32)

    inT_r = inT.ap().rearrange("(k p) t -> p k t", p=P)
    wT_r = wT.ap().rearrange("(k p) o -> p k o", p=P)
    outT_r = outT.ap().rearrange("(o p) t -> p o t", p=P)

    WG = 2 if KT % 2 == 0 else 1   # k-tiles per Sign-activation slice
    # W DMA schedule: small first load so the first stationary tile (and the
    # first matmul) is ready a few us in; bigger loads amortize DMA overhead.
    if KT == 16:
        WSCHED = (2, 2, 4, 4, 4)
    else:
        WSCHED = (KT,)
    NWQ = len(WSCHED)
    WQMAX = max(WSCHED)

    with tile.TileContext(nc) as tc:
        with (
            tc.tile_pool(name="const", bufs=1) as const,
            tc.tile_pool(name="wpool", bufs=1) as wpool,
            tc.tile_pool(name="wstream", bufs=2) as wstream,
            tc.tile_pool(name="small", bufs=1) as small,
            tc.tile_pool(name="inpool", bufs=28) as inpool,
            tc.tile_pool(name="outpool", bufs=2) as outpool,
            tc.tile_pool(name="pmm", bufs=8, space="PSUM") as pmm,
        ):
            bias_sb = const.tile([P, OT], f32)
            nc.gpsimd.dma_start(bias_sb[:], bias2d.ap())

            # PE clock warmup: the HAM gate holds the array at 1.2 GHz until
            # ~3.4us of sustained activity. Burn that window on throwaway
            # matmuls over a zeroed tile while the first weights stream in,
            # so the real matmuls start at 2.4 GHz.
            warm_src = const.tile([P, 256], bf16)
            nc.vector.memset(warm_src[:], 0.0)
            warm_ps = pmm.tile([P, 512], f32, tag="mm", name="warm_ps")
            NWARM = 14
            for wmm in range(NWARM):
                nc.tensor.matmul(warm_ps[0:16, 0:256], warm_src[:, 0:16],
                                 warm_src[:],
                                 start=(wmm == 0), stop=(wmm == NWARM - 1))

            # --- weight shard: sign -> bf16, |W| partial sums ---
            # Sign on ACT; |.| row-sums on DVE (reduce with absolute value);
            # no PE involvement anywhere in the scale chain so the in-order
            # PE queue is never blocked on it.
            sT = wpool.tile([P, KT, OSH], fp8)
            absacc = wpool.tile([P, NWQ], f32)
            k0 = 0
            for g, wq in enumerate(WSCHED):
                wt = wstream.tile([P, WQMAX, OSH], bf16, tag="wt",
                                  name=f"wt{g}")
                nc.sync.dma_start(
                    wt[:, :wq, :], wT_r[:, k0:k0 + wq, :]
                )
                for s in range(0, wq, WG):
                    sl = min(WG, wq - s)
                    nc.scalar.activation(sT[:, k0 + s:k0 + s + sl, :],
                                         wt[:, s:s + sl, :], AF.Sign)
                nc.vector.tensor_reduce(absacc[:, g:g + 1], wt[:, :wq, :],
                                        axis=mybir.AxisListType.XY,
                                        op=mybir.AluOpType.add,
                                        apply_absolute_value=True)
                k0 += wq

            # --- global scale via AllReduce of the scalar partial ---
            colsum = small.tile([P, 1], f32)
            nc.vector.reduce_sum(colsum[:], absacc[:], axis=mybir.AxisListType.X)
            # cross-partition gather via a DRAM bounce (partition axis can't
            # fold into an SBUF free axis) + free-axis reduce
            nc.gpsimd.dma_start(colsum_dram.ap(), colsum[:, 0])
            rowt = small.tile([1, P], f32)
            nc.gpsimd.dma_start(rowt[0:1, :], colsum_dram.ap()[None, :])
            part = small.tile([1, 8], f32)
            nc.vector.memset(part[:], 0.0)
            nc.vector.reduce_sum(part[0:1, 0:1], rowt[0:1, :],
                                 axis=mybir.AxisListType.X)
            # keep the scale chain's DMAs off the SP HWDGE ring: tot8 waits
            # on the collective, and the SP ring is FIFO — it would
            # head-of-line block every subsequent input load.
            nc.gpsimd.dma_start(cc_in.ap(), part[:])
            if use_collective:
                nc.gpsimd.collective_compute(
                    "AllReduce",
                    mybir.AluOpType.add,
                    replica_groups=[list(range(NCORES))],
                    ins=[cc_in.ap()],
                    outs=[cc_out.ap()],
                )
                cc_result = cc_out
            else:
                # timing-model variant (TimelineSim can't model collectives):
                # local partial stands in for the global sum
                nc.gpsimd.dma_start(cc_out.ap(), cc_in.ap())
                cc_result = cc_out
            # broadcast the reduced scalar to all 128 partitions straight
            # from DRAM (step-0 source AP)
            scale_raw = small.tile([P, 1], f32)
            with nc.allow_non_contiguous_dma(reason="scale broadcast"):
                nc.gpsimd.dma_start(scale_raw[:, 0:1],
                                    cc_result.ap()[0:1, 0:1].to_broadcast((P, 1)))
            scale_b = small.tile([P, 1], f32)
            nc.scalar.activation(scale_b[:], scale_raw[:], AF.Copy,
                                 scale=1.0 / float(D_OUT * D_IN))

            # --- main GEMM: outT[o, t] = sum_k sT[k, o] * inT[k, t] ---
            # ramped token spans: tiny first spans use 1 PSUM bank per
            # o-group so up to 7 o-groups accumulate k-incrementally while
            # the first weights/inputs are still arriving from HBM.
            spans = []
            t0 = 0
            for tq in SPAN_SCHEDULE:
                spans.append((t0, tq))
                t0 += tq
            assert t0 == TOKENS
            # repeat>1 re-runs the whole GEMM (same outputs rewritten) so a
            # wall-clock slope over R cancels fixed launch/proxy overheads.
            spans = [(q + r * len(spans), t0, tq)
                     for r in range(repeat)
                     for q, (t0, tq) in enumerate(spans)]
            nspans0 = len(SPAN_SCHEDULE)
            for q, t0, tq in spans:
                ncht = tq // 512
                inq = []
                for k in range(KT):
                    it = inpool.tile([P, TQ], bf16, tag="in",
                                     name=f"in_q{q}_k{k}")
                    nc.sync.dma_start(it[:, :tq], inT_r[:, k, t0:t0 + tq])
                    inq.append(it)
                for o in range(OT):
                    psums = [
                        pmm.tile([P, 512], f32, tag="mm", name=f"pp{q}_{o}_{c}")
                        for c in range(ncht)
                    ]
                    for k in range(KT):
                        lhsT = sT[:, k, o * P:(o + 1) * P]
                        for c in range(ncht):
                            nc.tensor.matmul(
                                psums[c][:], lhsT,
                                inq[k][:, c * 512:(c + 1) * 512],
                                start=(k == 0), stop=(k == KT - 1),
                            )
                    stage = outpool.tile([P, tq], f32, tag=f"stage{tq}",
                                         bufs=(8 if tq <= 512 else 2),
                                         name=f"st{q}_{o}")
                    if q % nspans0 < 3 and q < nspans0:
                        # early spans: scale may still be in flight (the
                        # AllReduce) — evict with a plain copy so the PSUM
                        # bank frees immediately, fold scale+bias in a
                        # second DVE pass before the store.
                        for c in range(ncht):
                            nc.scalar.activation(
                                stage[:, c * 512:(c + 1) * 512], psums[c][:],
                                AF.Copy)
                        nc.vector.tensor_scalar(
                            stage[:], stage[:],
                            scale_b[:, 0:1], bias_sb[:, o:o + 1],
                            mybir.AluOpType.mult, mybir.AluOpType.add)
                    elif q == len(spans) - 1 and o == OT - 1:
                        # very last tile: store per chunk so the final DMA
                        # isn't serialized behind all four evictions
                        for c in range(ncht):
                            nc.scalar.activation(
                                stage[:, c * 512:(c + 1) * 512], psums[c][:],
                                AF.Identity,
                                bias=bias_sb[:, o:o + 1], scale=scale_b[:, 0:1],
                            )
                            nc.scalar.dma_start(
                                outT_r[:, o, t0 + c * 512:t0 + (c + 1) * 512],
                                stage[:, c * 512:(c + 1) * 512])
                        continue
                    else:
                        for c in range(ncht):
                            nc.scalar.activation(
                                stage[:, c * 512:(c + 1) * 512], psums[c][:],
                                AF.Identity,
                                bias=bias_sb[:, o:o + 1], scale=scale_b[:, 0:1],
                            )
                    nc.scalar.dma_start(outT_r[:, o, t0:t0 + tq],
                                      stage[:])

    if dedup_ldw:
        _dedup_ldweights(nc, mybir)
    nc.compile()
    return nc


def _dedup_ldweights(nc, mybir):
    """Drop consecutive InstLdweights that reload the exact same stationary
    AP with only matmuls in between. Tile emits one weight load per matmul
    even when ncht matmuls share a stationary; on HW the redundant loads are
    partially exposed. The following non-self-loading matmuls keep using the
    already-loaded array state. Only waitless/updateless loads are removed."""
    removed = 0
    for bb in nc.m.functions[0].blocks:
        il = bb.instructions
        kept = []
        prev_sig = None
        for i in il:
            if isinstance(i, mybir.InstLdweights):
                sig = str(i.ins[0])
                if (sig == prev_sig and not i.has_wait()
                        and not i.has_update()):
                    nc.inst_map.pop(i.name, None)
                    removed += 1
                    continue
                prev_sig = sig
            elif isinstance(i, mybir.InstMatmult):
                pass
            elif getattr(i, "engine", None) == mybir.EngineType.PE:
                prev_sig = None
            kept.append(i)
        il[:] = kept


def _get_nc():
    if "nc" not in _NC_CACHE:
        _NC_CACHE["nc"] = _build_nc()
    return _NC_CACHE["nc"]


def _make_in_maps(input, weight, bias):
    inT = np.ascontiguousarray(input.T).astype(ml_dtypes.bfloat16)
    wT_full = weight.T  # [D_IN, D_OUT] view
    in_maps = []
    for j in range(NCORES):
        bsh = bias[j * OSH:(j + 1) * OSH]
        in_maps.append({
            "inT": inT,
            "wT": np.ascontiguousarray(
                wT_full[:, j * OSH:(j + 1) * OSH]).astype(ml_dtypes.bfloat16),
            "bias2d": np.ascontiguousarray(
                bsh.reshape(OT, P).T, dtype=np.float32),
        })
    return in_maps


def run(input, weight, bias, trace=False, **spmd_kwargs):
    from concourse.bass_utils import run_bass_kernel_spmd

    nc = _get_nc()
    in_maps = _make_in_maps(np.asarray(input, dtype=np.float32),
                            np.asarray(weight, dtype=np.float32),
                            np.asarray(bias, dtype=np.float32))
    res = run_bass_kernel_spmd(nc, in_maps, core_ids=list(range(NCORES)),
                               trace=trace, **spmd_kwargs)
    outT = np.concatenate([r["outT"] for r in res.results], axis=0)
    out = np.ascontiguousarray(outT.T)
    return out, res


def kernel(input, weight, bias):
    out, _ = run(input, weight, bias, trace=False)
    return out


# revision 51
# speedup vs baseline: 1.0015x; 1.0015x over previous
"""BitLinear (BitNet-style) kernel for 8 Trainium2 NeuronCores.

Computes: out = input @ (sign(W) * mean(|W|)).T + bias
  input [8192, 2048] f32, W [8192, 2048] f32, bias [8192] f32 -> out [8192, 8192] f32

Sharding: column-parallel over out_features. Core j owns W rows
[j*1024, (j+1)*1024). Each core computes sign() on its shard (scalar
engine) and a local |W| partial sum (vector engine reduce with absolute
value); partial sums are AllReduce'd across the 8 cores so the scale is
the global abs-mean. The GEMM runs in bf16 (sign(W) is exactly
representable; input/weights are rounded host-side), accumulating in
fp32 PSUM. scale (fp32) and bias (fp32) are fused into the PSUM->SBUF
eviction: out = psum * scale + bias.

Layout: host ships input already transposed (inT = input.T, bf16) and
the weight shard transposed (wT = W.T shard, bf16) so both GEMM operands
are K-major as the tensor engine requires; each core writes its out.T
shard [1024, 8192] contiguously and the host re-transposes once.

Perf notes (cost-model + real-HW repeat-slope profiled):
- 2048 matmuls of [K=128]x[M=128 o]x[N=512 t] stream at ~214 ns each —
  the bf16 1-col/cycle floor (~438 us busy); projection ~450 us/core.
- Stationary sign-weights are fp8e4 (+-1 exact): on real HW this removed
  ~90 us/iter of exposed LDWEIGHTS time vs a bf16 stationary (measured
  539 -> 446 us/iter via R-repeat wall-clock slope), since every matmul
  carries its own weight load and bf16 FWL loads don't fully hide.
- Output stores issue on the ACT HWDGE ring so they can't head-of-line
  block the next span's input loads on the SP ring (strict per-ring FIFO).
- The scale chain never touches the in-order PE queue (cross-partition
  sum via DRAM bounce + DVE, broadcast via step-0 DMA), and its small
  DMAs stay off the SP HWDGE FIFO so they can't head-of-line block the
  input loads while waiting on the collective.
- Ramped token spans (512,512,1024,2048x3): early spans use 1 PSUM bank
  per o-group (up to 8 in flight) and a copy-only eviction with the
  scale/bias folded in a second DVE pass, so nothing stalls on the
  AllReduce latency.
"""

import sys

for _p in ("/opt/trn_rl_repo",):
    if _p not in sys.path:
        sys.path.append(_p)

import ml_dtypes
import numpy as np

TOKENS = 8192
D_IN = 2048
D_OUT = 8192
NCORES = 8
OSH = D_OUT // NCORES  # 1024 out features per core
P = 128
KT = D_IN // P         # 16 k-tiles of 128
TQ = 2048              # resident token span
OT = OSH // P          # 8 o-tiles per core
SPAN_SCHEDULE = (512, 512, 1024, 2048, 2048, 2048)

_NC_CACHE = {}


def _build_nc(use_collective=True, repeat=1, dedup_ldw=True):
    import concourse.mybir as mybir
    import concourse.tile as tile
    from concourse import bacc

    f32 = mybir.dt.float32
    bf16 = mybir.dt.bfloat16
    fp8 = mybir.dt.float8e4
    AF = mybir.ActivationFunctionType

    nc = bacc.Bacc("TRN2", target_bir_lowering=False, debug=False,
                   num_devices=NCORES)

    inT = nc.dram_tensor("inT", [D_IN, TOKENS], bf16, kind="ExternalInput")
    wT = nc.dram_tensor("wT", [D_IN, OSH], bf16, kind="ExternalInput")
    bias2d = nc.dram_tensor("bias2d", [P, OT], f32, kind="ExternalInput")
    outT = nc.dram_tensor("outT", [OSH, TOKENS], f32, kind="ExternalOutput")
    cc_in = nc.dram_tensor("cc_in", [1, 8], f32)
    cc_out = nc.dram_tensor("cc_out", [1, 8], f32, addr_space="Shared")
    colsum_dram = nc.dram_tensor("colsum_dram", [P], f32)

    inT_r = inT.ap().rearrange("(k p) t -> p k t", p=P)
    wT_r = wT.ap().rearrange("(k p) o -> p k o", p=P)
    outT_r = outT.ap().rearrange("(o p) t -> p o t", p=P)

    WG = 2 if KT % 2 == 0 else 1   # k-tiles per Sign-activation slice
    # W DMA schedule: small first load so the first stationary tile (and the
    # first matmul) is ready a few us in; bigger loads amortize DMA overhead.
    if KT == 16:
        WSCHED = (2, 2, 4, 4, 4)
    else:
        WSCHED = (KT,)
    NWQ = len(WSCHED)
    WQMAX = max(WSCHED)

    with tile.TileContext(nc) as tc:
        with (
            tc.tile_pool(name="const", bufs=1) as const,
            tc.tile_pool(name="wpool", bufs=1) as wpool,
            tc.tile_pool(name="wstream", bufs=2) as wstream,
            tc.tile_pool(name="small", bufs=1) as small,
            tc.tile_pool(name="inpool", bufs=28) as inpool,
            tc.tile_pool(name="outpool", bufs=2) as outpool,
            tc.tile_pool(name="pmm", bufs=8, space="PSUM") as pmm,
        ):
            bias_sb = const.tile([P, OT], f32)
            nc.gpsimd.dma_start(bias_sb[:], bias2d.ap())

            # PE clock warmup: the HAM gate holds the array at 1.2 GHz until
            # ~3.4us of sustained activity. Burn that window on throwaway
            # matmuls over a zeroed tile while the first weights stream in,
            # so the real matmuls start at 2.4 GHz.
            warm_src = const.tile([P, 256], bf16)
            nc.vector.memset(warm_src[:], 0.0)
            warm_ps = pmm.tile([P, 512], f32, tag="mm", name="warm_ps")
            NWARM = 14
            for wmm in range(NWARM):
                nc.tensor.matmul(warm_ps[0:16, 0:256], warm_src[:, 0:16],
                                 warm_src[:],
                                 start=(wmm == 0), stop=(wmm == NWARM - 1))

            # --- weight shard: sign -> bf16, |W| partial sums ---
            # Sign on ACT; |.| row-sums on DVE (reduce with absolute value);
            # no PE involvement anywhere in the scale chain so the in-order
            # PE queue is never blocked on it.
            sT = wpool.tile([P, KT, OSH], fp8)
            absacc = wpool.tile([P, NWQ], f32)
            k0 = 0
            for g, wq in enumerate(WSCHED):
                wt = wstream.tile([P, WQMAX, OSH], bf16, tag="wt",
                                  name=f"wt{g}")
                nc.sync.dma_start(
                    wt[:, :wq, :], wT_r[:, k0:k0 + wq, :]
                )
                for s in range(0, wq, WG):
                    sl = min(WG, wq - s)
                    nc.scalar.activation(sT[:, k0 + s:k0 + s + sl, :],
                                         wt[:, s:s + sl, :], AF.Sign)
                nc.vector.tensor_reduce(absacc[:, g:g + 1], wt[:, :wq, :],
                                        axis=mybir.AxisListType.XY,
                                        op=mybir.AluOpType.add,
                                        apply_absolute_value=True)
                k0 += wq

            # --- global scale via AllReduce of the scalar partial ---
            colsum = small.tile([P, 1], f32)
            nc.vector.reduce_sum(colsum[:], absacc[:], axis=mybir.AxisListType.X)
            # cross-partition gather via a DRAM bounce (partition axis can't
            # fold into an SBUF free axis) + free-axis reduce
            nc.gpsimd.dma_start(colsum_dram.ap(), colsum[:, 0])
            rowt = small.tile([1, P], f32)
            nc.gpsimd.dma_start(rowt[0:1, :], colsum_dram.ap()[None, :])
            part = small.tile([1, 8], f32)
            nc.vector.memset(part[:], 0.0)
            nc.vector.reduce_sum(part[0:1, 0:1], rowt[0:1, :],
                                 axis=mybir.AxisListType.X)
            # keep the scale chain's DMAs off the SP HWDGE ring: tot8 waits
            # on the collective, and the SP ring is FIFO — it would
            # head-of-line block every subsequent input load.
            nc.gpsimd.dma_start(cc_in.ap(), part[:])
            if use_collective:
                nc.gpsimd.collective_compute(
                    "AllReduce",
                    mybir.AluOpType.add,
                    replica_groups=[list(range(NCORES))],
                    ins=[cc_in.ap()],
                    outs=[cc_out.ap()],
                )
                cc_result = cc_out
            else:
                # timing-model variant (TimelineSim can't model collectives):
                # local partial stands in for the global sum
                nc.gpsimd.dma_start(cc_out.ap(), cc_in.ap())
                cc_result = cc_out
            # broadcast the reduced scalar to all 128 partitions straight
            # from DRAM (step-0 source AP)
            scale_raw = small.tile([P, 1], f32)
            with nc.allow_non_contiguous_dma(reason="scale broadcast"):
                nc.gpsimd.dma_start(scale_raw[:, 0:1],
                                    cc_result.ap()[0:1, 0:1].to_broadcast((P, 1)))
            scale_b = small.tile([P, 1], f32)
            nc.scalar.activation(scale_b[:], scale_raw[:], AF.Copy,
                                 scale=1.0 / float(D_OUT * D_IN))

            # --- main GEMM: outT[o, t] = sum_k sT[k, o] * inT[k, t] ---
            # ramped token spans: tiny first spans use 1 PSUM bank per
            # o-group so up to 7 o-groups accumulate k-incrementally while
            # the first weights/inputs are still arriving from HBM.
            spans = []
            t0 = 0
            for tq in SPAN_SCHEDULE:
                spans.append((t0, tq))
                t0 += tq
            assert t0 == TOKENS
            # repeat>1 re-runs the whole GEMM (same outputs rewritten) so a
            # wall-clock slope over R cancels fixed launch/proxy overheads.
            spans = [(q + r * len(spans), t0, tq)
                     for r in range(repeat)
                     for q, (t0, tq) in enumerate(spans)]
            nspans0 = len(SPAN_SCHEDULE)
            for q, t0, tq in spans:
                ncht = tq // 512
                inq = []
                for k in range(KT):
                    it = inpool.tile([P, TQ], bf16, tag="in",
                                     name=f"in_q{q}_k{k}")
                    nc.sync.dma_start(it[:, :tq], inT_r[:, k, t0:t0 + tq])
                    inq.append(it)
                for o in range(OT):
                    psums = [
                        pmm.tile([P, 512], f32, tag="mm", name=f"pp{q}_{o}_{c}")
                        for c in range(ncht)
                    ]
                    for k in range(KT):
                        lhsT = sT[:, k, o * P:(o + 1) * P]
                        for c in range(ncht):
                            nc.tensor.matmul(
                                psums[c][:], lhsT,
                                inq[k][:, c * 512:(c + 1) * 512],
                                start=(k == 0), stop=(k == KT - 1),
                            )
                    stage = outpool.tile([P, tq], f32, tag=f"stage{tq}",
                                         bufs=(8 if tq <= 512 else 2),
                                         name=f"st{q}_{o}")
                    if q % nspans0 < 3 and q < nspans0:
                        # early spans: scale may still be in flight (the
                        # AllReduce) — evict with a plain copy so the PSUM
                        # bank frees immediately, fold scale+bias in a
                        # second DVE pass before the store.
                        for c in range(ncht):
                            nc.scalar.activation(
                                stage[:, c * 512:(c + 1) * 512], psums[c][:],
                                AF.Copy)
                        nc.vector.tensor_scalar(
                            stage[:], stage[:],
                            scale_b[:, 0:1], bias_sb[:, o:o + 1],
                            mybir.AluOpType.mult, mybir.AluOpType.add)
                    elif q == len(spans) - 1 and o == OT - 1:
                        # very last tile: store per chunk so the final DMA
                        # isn't serialized behind all four evictions
                        for c in range(ncht):
                            nc.scalar.activation(
                                stage[:, c * 512:(c + 1) * 512], psums[c][:],
                                AF.Identity,
                                bias=bias_sb[:, o:o + 1], scale=scale_b[:, 0:1],
                            )
                            eng = nc.scalar if c % 2 == 0 else nc.sync
                            eng.dma_start(
                                outT_r[:, o, t0 + c * 512:t0 + (c + 1) * 512],
                                stage[:, c * 512:(c + 1) * 512])
                        continue
                    else:
                        for c in range(ncht):
                            nc.scalar.activation(
                                stage[:, c * 512:(c + 1) * 512], psums[c][:],
                                AF.Identity,
                                bias=bias_sb[:, o:o + 1], scale=scale_b[:, 0:1],
                            )
                    nc.scalar.dma_start(outT_r[:, o, t0:t0 + tq],
                                      stage[:])

    if dedup_ldw:
        _dedup_ldweights(nc, mybir)
    nc.compile()
    return nc


def _dedup_ldweights(nc, mybir):
    """Drop consecutive InstLdweights that reload the exact same stationary
    AP with only matmuls in between. Tile emits one weight load per matmul
    even when ncht matmuls share a stationary; on HW the redundant loads are
    partially exposed. The following non-self-loading matmuls keep using the
    already-loaded array state. Only waitless/updateless loads are removed."""
    removed = 0
    for bb in nc.m.functions[0].blocks:
        il = bb.instructions
        kept = []
        prev_sig = None
        for i in il:
            if isinstance(i, mybir.InstLdweights):
                sig = str(i.ins[0])
                if (sig == prev_sig and not i.has_wait()
                        and not i.has_update()):
                    nc.inst_map.pop(i.name, None)
                    removed += 1
                    continue
                prev_sig = sig
            elif isinstance(i, mybir.InstMatmult):
                pass
            elif getattr(i, "engine", None) == mybir.EngineType.PE:
                prev_sig = None
            kept.append(i)
        il[:] = kept


def _get_nc():
    if "nc" not in _NC_CACHE:
        _NC_CACHE["nc"] = _build_nc()
    return _NC_CACHE["nc"]


def _make_in_maps(input, weight, bias):
    inT = np.ascontiguousarray(input.T).astype(ml_dtypes.bfloat16)
    wT_full = weight.T  # [D_IN, D_OUT] view
    in_maps = []
    for j in range(NCORES):
        bsh = bias[j * OSH:(j + 1) * OSH]
        in_maps.append({
            "inT": inT,
            "wT": np.ascontiguousarray(
                wT_full[:, j * OSH:(j + 1) * OSH]).astype(ml_dtypes.bfloat16),
            "bias2d": np.ascontiguousarray(
                bsh.reshape(OT, P).T, dtype=np.float32),
        })
    return in_maps


def run(input, weight, bias, trace=False, **spmd_kwargs):
    from concourse.bass_utils import run_bass_kernel_spmd

    nc = _get_nc()
    in_maps = _make_in_maps(np.asarray(input, dtype=np.float32),
                            np.asarray(weight, dtype=np.float32),
                            np.asarray(bias, dtype=np.float32))
    res = run_bass_kernel_spmd(nc, in_maps, core_ids=list(range(NCORES)),
                               trace=trace, **spmd_kwargs)
    outT = np.concatenate([r["outT"] for r in res.results], axis=0)
    out = np.ascontiguousarray(outT.T)
    return out, res


def kernel(input, weight, bias):
    out, _ = run(input, weight, bias, trace=False)
    return out
